# revision 1
# baseline (speedup 1.0000x reference)
"""Trainium2 Bass kernel for a 2-layer autoregressive LSTM (teacher-forced)
with zoneout (eval mode), conditioning input, and output projection.

Strategy (8 NeuronCores, one TRN2 chip):
  - Model-parallel over the 4*H=4096 gate dimension: core k owns hidden units
    [k*128, (k+1)*128) of each gate (i, f, o, g) for BOTH layers, full batch.
  - Per timestep each core computes its 512 gate rows with gate-stationary
    [128,128] matmul tiles (rhs = h^T [128, 32]), the LSTM cell elementwise on
    [128, 32] tiles, then all cores AllGather their 128-unit h slices so
    everyone has the full h for the next step.
  - Input-side products are hoisted off the serial chain and batched over
    L-step chunks: U0 = xin @ W_ih0^T (xin known ahead of time),
    U1 = h0 @ W_ih1^T (lagging layer 1 by LAG steps), y = h1 @ proj^T
    (proj split 10 output rows per core).  U terms enter the per-step PSUM
    accumulation through an identity-weight matmul; biases ride along as a
    constant-one feature row.
"""

import numpy as np

import concourse.bass as bass
import concourse.bacc as bacc
import concourse.tile as tile
from concourse import bass_utils, mybir

AF = mybir.ActivationFunctionType
ALU = mybir.AluOpType

# Problem constants
B, D, T_FULL, C, H = 32, 80, 1000, 512, 1024
ZONEOUT = 0.1

# Kernel layout constants
P = 128          # partitions
NC = 8           # cores
HU = H // NC     # hidden units per core = 128
MT = 4           # gate m-tiles per core (i, f, o, g)
KIN = 5          # xin contraction chunks (592+bias padded to 640 = 5*128)
KH = H // P      # h contraction chunks = 8
L = 16           # time-chunk length for the batched GEMMs
LAG = L + 4      # layer-1 lag behind layer 0
PJ = D // NC     # proj rows per core = 10
PJP = 16         # padded proj rows per core

BF16 = mybir.dt.bfloat16
F32 = mybir.dt.float32
NP_BF16 = mybir.dt.np(BF16)

RG = [list(range(NC))]


def _chunks(T):
    n = (T + L - 1) // L
    return [(c, min(L, T - c * L)) for c in range(n)]


def build_nc(T):
    """Build the SPMD Bass program for sequence length T."""
    TB = T * B
    nc = bacc.Bacc(
        "TRN2",
        target_bir_lowering=False,
        debug=False,
        enable_asserts=False,
        num_devices=NC,
    )

    # ---- I/O ----
    xinT_d = nc.dram_tensor("xinT", [P, KIN, TB], BF16, kind="ExternalInput")
    w0T_d = nc.dram_tensor("w0T", [P, KIN, MT, P], BF16, kind="ExternalInput")
    wh0T_d = nc.dram_tensor("wh0T", [P, KH, MT, P], BF16, kind="ExternalInput")
    w1T_d = nc.dram_tensor("w1T", [P, KH + 1, MT, P], BF16, kind="ExternalInput")
    wh1T_d = nc.dram_tensor("wh1T", [P, KH, MT, P], BF16, kind="ExternalInput")
    pjT_d = nc.dram_tensor("pjT", [P, KH + 1, PJP], BF16, kind="ExternalInput")
    id_d = nc.dram_tensor("ident", [P, P], BF16, kind="ExternalInput")
    y_d = nc.dram_tensor("y_out", [PJP, TB], F32, kind="ExternalOutput")

    ch = _chunks(T)
    nch = len(ch)
    # chunk emission schedules: iteration t -> chunk index
    u0_at = {(c - 1) * L: c for c, _ in ch if c >= 1}
    u1_at = {c * L + lc: c for c, lc in ch}
    pj_at = {c * L + lc + LAG: c for c, lc in ch}

    with tile.TileContext(nc) as tc:
        with (
            tc.tile_pool(name="const", bufs=1) as cp,
            tc.tile_pool(name="work", bufs=3) as wp,
            tc.tile_pool(name="dram", bufs=3, space="DRAM") as dp,
            tc.tile_pool(name="ps0", bufs=2, space="PSUM") as ps0p,
            tc.tile_pool(name="ps1", bufs=2, space="PSUM") as ps1p,
            tc.tile_pool(name="psu", bufs=2, space="PSUM") as psup,
            tc.tile_pool(name="psp", bufs=1, space="PSUM") as pspp,
        ):
            # resident tiles
            w0_sb = cp.tile([P, KIN, MT, P], BF16)
            wh0_sb = cp.tile([P, KH, MT, P], BF16)
            w1_sb = cp.tile([P, KH + 1, MT, P], BF16)
            wh1_sb = cp.tile([P, KH, MT, P], BF16)
            pj_sb = cp.tile([P, KH + 1, PJP], BF16)
            id_sb = cp.tile([P, P], BF16)
            h0_hist = cp.tile([P, KH + 1, 2 * L, B], BF16)
            h1_hist = cp.tile([P, KH + 1, 2 * L, B], BF16)
            U0_sb = cp.tile([P, 2, MT, L * B], BF16)
            U1_sb = cp.tile([P, 2, MT, L * B], BF16)
            c0_t = cp.tile([P, B], F32)
            h0_t = cp.tile([P, B], F32)
            c1_t = cp.tile([P, B], F32)
            h1_t = cp.tile([P, B], F32)

            nc.sync.dma_start(w0_sb[:], w0T_d[:])
            nc.sync.dma_start(wh0_sb[:], wh0T_d[:])
            nc.sync.dma_start(w1_sb[:], w1T_d[:])
            nc.sync.dma_start(wh1_sb[:], wh1T_d[:])
            nc.sync.dma_start(pj_sb[:], pjT_d[:])
            nc.sync.dma_start(id_sb[:], id_d[:])

            nc.vector.memset(h0_hist[:], 0.0)
            nc.vector.memset(h1_hist[:], 0.0)
            nc.vector.memset(h0_hist[:, KH, :, :], 1.0)  # bias ones-row block
            nc.vector.memset(h1_hist[:, KH, :, :], 1.0)
            nc.vector.memset(c0_t[:], 0.0)
            nc.vector.memset(h0_t[:], 0.0)
            nc.vector.memset(c1_t[:], 0.0)
            nc.vector.memset(h1_t[:], 0.0)

            def emit_u0(c):
                lc = ch[c][1]
                nco = lc * B
                xin_t = wp.tile([P, KIN, L * B], BF16, tag="xin")
                nc.sync.dma_start(
                    xin_t[:, :, :nco], xinT_d[:, :, c * L * B : c * L * B + nco]
                )
                for m in range(MT):
                    pt = psup.tile([P, L * B], F32, tag="psu")
                    for k in range(KIN):
                        nc.tensor.matmul(
                            pt[:, :nco],
                            w0_sb[:, k, m, :],
                            xin_t[:, k, :nco],
                            start=(k == 0),
                            stop=(k == KIN - 1),
                        )
                    nc.vector.tensor_copy(U0_sb[:, c % 2, m, :nco], pt[:, :nco])

            def emit_u1(c):
                lc = ch[c][1]
                nco = lc * B
                half = (c % 2) * L
                for m in range(MT):
                    pt = psup.tile([P, L * B], F32, tag="psu")
                    for k in range(KH + 1):
                        rhs = h0_hist[:, k, half : half + lc, :].rearrange(
                            "p l b -> p (l b)"
                        )
                        nc.tensor.matmul(
                            pt[:, :nco],
                            w1_sb[:, k, m, :],
                            rhs,
                            start=(k == 0),
                            stop=(k == KH),
                        )
                    nc.vector.tensor_copy(U1_sb[:, c % 2, m, :nco], pt[:, :nco])

            def emit_proj(c):
                lc = ch[c][1]
                nco = lc * B
                half = (c % 2) * L
                pt = pspp.tile([PJP, L * B], F32, tag="psp")
                for k in range(KH + 1):
                    rhs = h1_hist[:, k, half : half + lc, :].rearrange(
                        "p l b -> p (l b)"
                    )
                    nc.tensor.matmul(
                        pt[:, :nco],
                        pj_sb[:, k, :],
                        rhs,
                        start=(k == 0),
                        stop=(k == KH),
                    )
                y_t = wp.tile([PJP, L * B], F32, tag="ysb")
                nc.scalar.copy(y_t[:, :nco], pt[:, :nco])
                nc.sync.dma_start(y_d[:, c * L * B : c * L * B + nco], y_t[:, :nco])

            def cell(ell, t, send_t):
                hist = h0_hist if ell == 0 else h1_hist
                W = wh0_sb if ell == 0 else wh1_sb
                U = U0_sb if ell == 0 else U1_sb
                cst = c0_t if ell == 0 else c1_t
                hst = h0_t if ell == 0 else h1_t
                pool = ps0p if ell == 0 else ps1p
                ps = pool.tile([P, MT * B], F32, tag=f"ps{ell}")
                slot = (t - 1) % (2 * L)
                ci, si = t // L, t % L
                for m in range(MT):
                    o = ps[:, m * B : (m + 1) * B]
                    for k in range(KH):
                        nc.tensor.matmul(
                            o, W[:, k, m, :], hist[:, k, slot, :],
                            start=(k == 0), stop=False,
                        )
                    nc.tensor.matmul(
                        o, id_sb[:],
                        U[:, ci % 2, m, si * B : (si + 1) * B],
                        start=False, stop=True,
                    )
                S = wp.tile([P, 3 * B], F32, tag=f"S{ell}")
                nc.scalar.activation(S[:], ps[:, 0 : 3 * B], AF.Sigmoid)
                Tg = wp.tile([P, B], F32, tag=f"Tg{ell}")
                nc.scalar.activation(Tg[:], ps[:, 3 * B : 4 * B], AF.Tanh)
                c01 = wp.tile([P, B], F32, tag=f"c01{ell}")
                nc.vector.tensor_scalar_mul(c01[:], cst[:], 0.1)
                h01 = wp.tile([P, B], F32, tag=f"h01{ell}")
                nc.vector.tensor_scalar_mul(h01[:], hst[:], 0.1)
                So9 = wp.tile([P, B], F32, tag=f"So9{ell}")
                nc.vector.tensor_scalar_mul(So9[:], S[:, 2 * B : 3 * B], 0.9)
                R = wp.tile([P, B], F32, tag=f"R{ell}")
                nc.vector.tensor_mul(R[:], S[:, B : 2 * B], cst[:])
                Pi = wp.tile([P, B], F32, tag=f"Pi{ell}")
                nc.vector.tensor_mul(Pi[:], S[:, 0:B], Tg[:])
                cn = wp.tile([P, B], F32, tag=f"cn{ell}")
                nc.vector.tensor_add(cn[:], R[:], Pi[:])
                # c <- 0.9*c_new + 0.1*c_old
                nc.vector.scalar_tensor_tensor(
                    cst[:], cn[:], 0.9, c01[:], op0=ALU.mult, op1=ALU.add
                )
                Tc = wp.tile([P, B], F32, tag=f"Tc{ell}")
                nc.scalar.activation(Tc[:], cn[:], AF.Tanh)
                Hz = wp.tile([P, B], F32, tag=f"Hz{ell}")
                nc.vector.tensor_mul(Hz[:], So9[:], Tc[:])
                # h <- 0.9*o*tanh(c_new) + 0.1*h_old
                nc.vector.tensor_add(hst[:], Hz[:], h01[:])
                nc.scalar.copy(send_t[:, ell * B : (ell + 1) * B], hst[:])

            emit_u0(0)

            for t in range(T + LAG):
                send_t = wp.tile([P, 2 * B], BF16, tag="send")
                if t < T:
                    cell(0, t, send_t)
                else:
                    nc.vector.memset(send_t[:, 0:B], 0.0)
                tau = t - LAG
                if tau >= 0:
                    cell(1, tau, send_t)
                else:
                    nc.vector.memset(send_t[:, B : 2 * B], 0.0)

                agi = dp.tile([P, 2 * B], BF16, tag="agi")
                ago = dp.tile([NC * P, 2 * B], BF16, tag="ago")
                nc.sync.dma_start(agi[:], send_t[:])
                nc.gpsimd.collective_compute(
                    "AllGather",
                    ALU.bypass,
                    replica_groups=RG,
                    ins=[agi.opt()],
                    outs=[ago.opt()],
                )
                agov = ago[:].rearrange("(k p) b -> p k b", p=P)
                if t < T:
                    nc.sync.dma_start(
                        h0_hist[:, 0:KH, t % (2 * L), :], agov[:, :, 0:B]
                    )
                if tau >= 0:
                    nc.sync.dma_start(
                        h1_hist[:, 0:KH, tau % (2 * L), :], agov[:, :, B : 2 * B]
                    )

                if t in u0_at:
                    emit_u0(u0_at[t])
                if t in u1_at:
                    emit_u1(u1_at[t])
                if t in pj_at:
                    emit_proj(pj_at[t])

            # chunks scheduled past the last iteration
            for t_late in sorted(pj_at):
                if t_late >= T + LAG:
                    emit_proj(pj_at[t_late])

    nc.compile()
    return nc


# ---------------- host-side data prep ----------------

def _gate_rows(k):
    u = np.arange(k * HU, (k + 1) * HU)
    return np.concatenate([u, H + u, 3 * H + u, 2 * H + u])  # i, f, o, g


def _lhsT_blocks(w, nk, mt=MT):
    """w: [mt*P, nk*P] (already row-sliced/ordered) -> [P, nk, mt, P] lhsT tiles."""
    a = w.reshape(mt, P, nk, P)  # [m, j, k, p]
    return np.ascontiguousarray(a.transpose(3, 2, 0, 1))  # [p, k, m, j]


def prep_inputs(inputs, T):
    x = np.asarray(inputs["x"], np.float32)[:, :, :T]
    cond = np.asarray(inputs["cond"], np.float32)[:, :, :T]
    w_ih0 = np.asarray(inputs["w_ih0"], np.float32)
    w_hh0 = np.asarray(inputs["w_hh0"], np.float32)
    b0 = np.asarray(inputs["b_ih0"], np.float32) + np.asarray(inputs["b_hh0"], np.float32)
    w_ih1 = np.asarray(inputs["w_ih1"], np.float32)
    w_hh1 = np.asarray(inputs["w_hh1"], np.float32)
    b1 = np.asarray(inputs["b_ih1"], np.float32) + np.asarray(inputs["b_hh1"], np.float32)
    proj_w = np.asarray(inputs["proj_w"], np.float32)
    proj_b = np.asarray(inputs["proj_b"], np.float32)

    TB = T * B
    in0 = D + C
    xs = np.concatenate([np.zeros((B, D, 1), np.float32), x[:, :, : T - 1]], axis=2)
    xin = np.concatenate([xs, cond], axis=1)  # [B, 592, T]
    xin_pad = np.zeros((B, KIN * P, T), np.float32)
    xin_pad[:, :in0] = xin
    xin_pad[:, in0] = 1.0  # bias feature
    # [feat, T, B] -> [feat, TB] with col index t*B+b
    xinT = np.ascontiguousarray(xin_pad.transpose(1, 2, 0)).reshape(KIN * P, TB)
    xinT = np.ascontiguousarray(
        xinT.reshape(KIN, P, TB).transpose(1, 0, 2)
    ).astype(NP_BF16)

    w_ih0_pad = np.zeros((4 * H, KIN * P), np.float32)
    w_ih0_pad[:, :in0] = w_ih0
    w_ih0_pad[:, in0] = b0

    ident = np.eye(P, dtype=NP_BF16)

    in_maps = []
    for k in range(NC):
        r = _gate_rows(k)
        w0T = _lhsT_blocks(w_ih0_pad[r], KIN).astype(NP_BF16)
        wh0T = _lhsT_blocks(w_hh0[r], KH).astype(NP_BF16)
        w1_ext = np.zeros((MT * P, (KH + 1) * P), np.float32)
        w1_ext[:, : KH * P] = w_ih1[r]
        w1_ext[:, KH * P] = b1[r]  # ones-row bias block (row 0 of chunk KH)
        w1T = _lhsT_blocks(w1_ext, KH + 1).astype(NP_BF16)
        wh1T = _lhsT_blocks(w_hh1[r], KH).astype(NP_BF16)
        pjT = np.zeros((P, KH + 1, PJP), np.float32)
        rows = np.arange(k * PJ, (k + 1) * PJ)
        for kk in range(KH):
            pjT[:, kk, :PJ] = proj_w[rows, kk * P : (kk + 1) * P].T
        pjT[0, KH, :PJ] = proj_b[rows]
        in_maps.append(
            {
                "xinT": xinT,
                "w0T": w0T,
                "wh0T": wh0T,
                "w1T": w1T,
                "wh1T": wh1T,
                "pjT": pjT.astype(NP_BF16),
                "ident": ident,
            }
        )
    return in_maps


def assemble(results, x_lengths, T):
    y = np.concatenate([r["y_out"][:PJ] for r in results], axis=0)  # [80, TB]
    y = y.reshape(D, T, B).transpose(2, 0, 1)  # [B, D, T]
    lens = np.asarray(x_lengths).astype(np.int64)
    mask = (np.arange(T)[None, :] < lens[:, None]).astype(np.float32)
    return np.ascontiguousarray(y * mask[:, None, :])


_NC_CACHE = {}


def run(inputs, T=T_FULL, trace=False, **kw):
    if T not in _NC_CACHE:
        _NC_CACHE[T] = build_nc(T)
    nc = _NC_CACHE[T]
    in_maps = prep_inputs(inputs, T)
    res = bass_utils.run_bass_kernel_spmd(
        nc, in_maps, core_ids=list(range(NC)), trace=trace, **kw
    )
    out = assemble(res.results, inputs["x_lengths"], T)
    return out, res


def kernel(**inputs) -> np.ndarray:
    out, _ = run(inputs, T=T_FULL)
    return out



# revision 2
# speedup vs baseline: 1.0092x; 1.0092x over previous
"""Trainium2 Bass kernel v2 for the 2-layer autoregressive LSTM.

Same math/distribution as the baseline (model-parallel over the 4*H gate
rows, 128 hidden units per core per layer), but the per-timestep AllGather of
h-slices is done with SBUF->SBUF remote_dma_broadcast (all 16 DMA engines,
~1-2us) instead of a DRAM-bounced ncfw collective (~100us+), and the whole
program is raw bass (no Tile scheduler) with explicit semaphore sync.

Per step t (layer 0; layer 1 runs the same pipeline LAG steps behind):
  PE:   gates = sum_k W_hh0[:,k] @ h_recv[k, t-1] + I @ U0[:, t]   (36 MMs)
  ACT:  S = sigmoid(gates_ifo), Tg = tanh(gates_g), later Tc = tanh(c_new)
  DVE:  c' = .9(S_f c + S_i Tg) + .1 c ; h' = .9 S_o Tc + .1 h ; send=bf16(h')
  Pool: trigger previously-prepped remote_dma_broadcast of send ->
        recv[:, my_id, slot, :] on all 8 cores; prep step t+1's broadcast.
U0 = W_ih0 @ xin (teacher-forced input, known ahead), U1 = W_ih1 @ h0 and
y = proj @ h1 are batched over L-step chunks off the critical path.
"""

import numpy as np

import concourse.bass as bass
import concourse.bacc as bacc
from concourse import bass_utils, mybir

AF = mybir.ActivationFunctionType
ALU = mybir.AluOpType

# Problem constants
B, D, T_FULL, C, H = 32, 80, 1000, 512, 1024

# Layout constants
P = 128
NC = 8
MT = 4            # gate m-tiles (i, f, o, g)
KIN = 5           # xin contraction chunks (592+bias -> 640)
KH = H // P       # 8
L = 16            # U/proj chunk length
LAG = L + 4
S = 2 * L         # hist slots
PJ = D // NC      # 10
PJP = 16

BF16 = mybir.dt.bfloat16
F32 = mybir.dt.float32
NP_BF16 = mybir.dt.np(BF16)


def _chunks(T):
    n = (T + L - 1) // L
    return [(c, min(L, T - c * L)) for c in range(n)]


class Dep:
    """Tiny dependency tracker: every sync-relevant instruction bumps its
    engine's completion sem; named values map reads/writes to (sem, count)
    deps; waits are emitted (and elided via per-engine watermarks)."""

    def __init__(self, nc):
        self.nc = nc
        self.engs = {
            "pe": nc.tensor,
            "act": nc.scalar,
            "dve": nc.vector,
            "gp": nc.gpsimd,
            "sp": nc.sync,
        }
        self.sem = {k: nc.alloc_semaphore(f"c_{k}") for k in self.engs}
        self.cnt = {k: 0 for k in self.engs}
        # extra named sems (dma, remote...) registered via add_sem
        self.extra = {}
        self.wm = {k: {} for k in self.engs}  # consumer -> {semnum: waited}
        self.lastw = {}   # name -> (semhandle, val)
        self.readers = {}  # name -> [(semhandle, val)]

    def add_sem(self, name):
        s = self.nc.alloc_semaphore(name)
        self.extra[name] = s
        return s

    def _deps_for(self, reads, writes, extra):
        deps = []
        for nm in reads:
            d = self.lastw.get(nm)
            if d:
                deps.append(d)
        for nm in writes:
            d = self.lastw.get(nm)
            if d:
                deps.append(d)
            deps.extend(self.readers.get(nm, ()))
        deps.extend(extra)
        merged = {}
        for sem, val in deps:
            k = sem.num
            if k not in merged or merged[k][1] < val:
                merged[k] = (sem, val)
        return merged

    def wait(self, eng, merged):
        wmk = self.wm[eng]
        e = self.engs[eng]
        for k, (sem, val) in merged.items():
            if wmk.get(k, -1) >= val:
                continue
            e.wait_ge(sem, val)
            wmk[k] = val

    def emit(self, eng, fn, reads=(), writes=(), extra=(), inc=True):
        """fn() must emit exactly one instruction and return BassInstruction."""
        merged = self._deps_for(reads, writes, extra)
        self.wait(eng, merged)
        ins = fn()
        if inc:
            self.cnt[eng] += 1
            ins.then_inc(self.sem[eng], 1)
        tok = (self.sem[eng], self.cnt[eng])
        for nm in writes:
            self.lastw[nm] = tok
            self.readers[nm] = []
        for nm in reads:
            self.readers.setdefault(nm, []).append(tok)
        return ins

    def group(self, eng, fns, reads=(), writes=(), extra=()):
        """Emit a group of instructions; only the last gets the inc (in-order
        completion per engine makes this sound)."""
        merged = self._deps_for(reads, writes, extra)
        self.wait(eng, merged)
        last = None
        for fn in fns:
            last = fn()
        self.cnt[eng] += 1
        last.then_inc(self.sem[eng], 1)
        tok = (self.sem[eng], self.cnt[eng])
        for nm in writes:
            self.lastw[nm] = tok
            self.readers[nm] = []
        for nm in reads:
            self.readers.setdefault(nm, []).append(tok)
        return last

    def note_remote_write(self, name, sem, val):
        self.lastw[name] = (sem, val)
        self.readers[name] = []

    def note_dma_read(self, name, sem, val):
        """A triggered DMA reads `name`; completion observable at sem>=val."""
        self.readers.setdefault(name, []).append((sem, val))


def build_nc(T):
    TB = T * B
    nc = bacc.Bacc(
        "TRN2",
        target_bir_lowering=False,
        debug=False,
        enable_asserts=False,
        num_devices=NC,
        detect_race_conditions=False,
    )

    # ---- I/O (same host-side layout as baseline) ----
    xinT_d = nc.dram_tensor("xinT", [P, KIN, TB], BF16, kind="ExternalInput")
    w0T_d = nc.dram_tensor("w0T", [P, KIN, MT, P], BF16, kind="ExternalInput")
    wh0T_d = nc.dram_tensor("wh0T", [P, KH, MT, P], BF16, kind="ExternalInput")
    w1T_d = nc.dram_tensor("w1T", [P, KH + 1, MT, P], BF16, kind="ExternalInput")
    wh1T_d = nc.dram_tensor("wh1T", [P, KH, MT, P], BF16, kind="ExternalInput")
    pjT_d = nc.dram_tensor("pjT", [P, KH + 1, PJP], BF16, kind="ExternalInput")
    id_d = nc.dram_tensor("ident", [P, P], BF16, kind="ExternalInput")
    y_d = nc.dram_tensor("y_out", [PJP, TB], F32, kind="ExternalOutput")

    ch = _chunks(T)
    u0_at = {(c - 1) * L: c for c, _ in ch if c >= 1}
    u1_at = {c * L + lc: c for c, lc in ch}
    pj_at = {c * L + lc + LAG: c for c, lc in ch}

    # ---- SBUF ----
    w0_sb = nc.alloc_sbuf_tensor("w0_sb", [P, KIN, MT, P], BF16)
    wh0_sb = nc.alloc_sbuf_tensor("wh0_sb", [P, KH, MT, P], BF16)
    w1_sb = nc.alloc_sbuf_tensor("w1_sb", [P, KH + 1, MT, P], BF16)
    wh1_sb = nc.alloc_sbuf_tensor("wh1_sb", [P, KH, MT, P], BF16)
    pj_sb = nc.alloc_sbuf_tensor("pj_sb", [P, KH + 1, PJP], BF16)
    id_sb = nc.alloc_sbuf_tensor("id_sb", [P, P], BF16)
    # combined hist: [P, chunk(KH+1), slot(S), 2B]; cols 0:B=h0, B:2B=h1;
    # chunk KH is the ones/bias block
    hh = nc.alloc_sbuf_tensor("hh", [P, KH + 1, S, 2 * B], BF16)
    xin_sb = nc.alloc_sbuf_tensor("xin_sb", [P, 2, KIN, L * B], BF16)
    U0_sb = nc.alloc_sbuf_tensor("U0_sb", [P, 2, MT, L * B], BF16)
    U1_sb = nc.alloc_sbuf_tensor("U1_sb", [P, 2, MT, L * B], BF16)
    ysb = nc.alloc_sbuf_tensor("ysb", [PJP, 2, L * B], F32)
    send0 = nc.alloc_sbuf_tensor("send0", [P, 3, 2 * B], BF16)
    c0_t = nc.alloc_sbuf_tensor("c0_t", [P, B], F32)
    h0_t = nc.alloc_sbuf_tensor("h0_t", [P, B], F32)
    c1_t = nc.alloc_sbuf_tensor("c1_t", [P, B], F32)
    h1_t = nc.alloc_sbuf_tensor("h1_t", [P, B], F32)
    # cell temps, parity-double
    S0s = nc.alloc_sbuf_tensor("S0s", [P, 2, 3 * B], F32)
    S1s = nc.alloc_sbuf_tensor("S1s", [P, 2, 3 * B], F32)
    Tg0s = nc.alloc_sbuf_tensor("Tg0s", [P, 2, B], F32)
    Tg1s = nc.alloc_sbuf_tensor("Tg1s", [P, 2, B], F32)
    cn0s = nc.alloc_sbuf_tensor("cn0s", [P, 2, B], F32)
    cn1s = nc.alloc_sbuf_tensor("cn1s", [P, 2, B], F32)
    Tc0s = nc.alloc_sbuf_tensor("Tc0s", [P, 2, B], F32)
    Tc1s = nc.alloc_sbuf_tensor("Tc1s", [P, 2, B], F32)
    tmpA = nc.alloc_sbuf_tensor("tmpA", [P, 8, B], F32)  # c01,h01,So9,R,Pi,Hz x2
    tmpB = nc.alloc_sbuf_tensor("tmpB", [P, 8, B], F32)

    # ---- PSUM (bank-granular allocations) ----
    ps0 = [nc.alloc_psum_tensor(f"ps0_{i}", [P, MT * B], F32) for i in range(2)]
    ps1 = [nc.alloc_psum_tensor(f"ps1_{i}", [P, MT * B], F32) for i in range(2)]
    psU = [nc.alloc_psum_tensor(f"psU_{i}", [P, L * B], F32) for i in range(2)]
    psP = nc.alloc_psum_tensor("psP", [PJP, L * B], F32)

    dep = Dep(nc)
    rsem0 = dep.add_sem("rsem0")
    rsem1 = rsem0
    lsem0 = dep.add_sem("lsem0")
    lsem1 = lsem0
    psem = dep.add_sem("psem")
    dmas = dep.add_sem("dmas")   # HWDGE load completions (x16)
    dmay = dep.add_sem("dmay")   # y store completions (x16)

    pe, act, dve, gp, sp = (
        nc.tensor, nc.scalar, nc.vector, nc.gpsimd, nc.sync)

    # ================= preamble =================
    ndma = 0

    def dma_in(dst_ap, src_ap, name):
        nonlocal ndma
        ndma += 1
        n = ndma
        sp.dma_start(dst_ap, src_ap).then_inc(dmas, 16)
        dep.note_remote_write(name, dmas, 16 * n)

    dma_in(w0_sb[:], w0T_d[:], "w0")
    dma_in(wh0_sb[:], wh0T_d[:], "wh0")
    dma_in(w1_sb[:], w1T_d[:], "w1")
    dma_in(wh1_sb[:], wh1T_d[:], "wh1")
    dma_in(pj_sb[:], pjT_d[:], "pj")
    dma_in(id_sb[:], id_d[:], "id")
    dma_in(xin_sb[:, 0, :, :], xinT_d[:, :, 0: L * B], "xin0")
    if len(ch) > 1:
        dma_in(xin_sb[:, 1, :, :], xinT_d[:, :, L * B: 2 * L * B], "xin1")

    # ones/bias blocks of the hists (local-only; remote writes never touch
    # chunk KH)
    dep.emit("dve", lambda: dve.memset(hh[:, KH, :, :], 1.0), writes=["h0ones"])
    dep.emit("dve", lambda: dve.memset(send0[:], 0.0),
             writes=["send00", "send01", "send02", "send10", "send11", "send12"])
    dep.lastw["h1ones"] = dep.lastw["h0ones"]
    dep.emit("dve", lambda: dve.memset(c0_t[:], 0.0), writes=["c0"])
    dep.emit("dve", lambda: dve.memset(h0_t[:], 0.0), writes=["h0"])
    dep.emit("dve", lambda: dve.memset(c1_t[:], 0.0), writes=["c1"])
    dep.emit("dve", lambda: dve.memset(h1_t[:], 0.0), writes=["h1"])

    # all peers entered (their NEFF loaded; remote writes safe)
    gp.bir_kernel_barrier_wait([[i for i in range(NC)]])
    pid = gp.partition_id()
    rdests = [(0, k) for k in range(NC)]

    nprep = 0      # runtime preps enqueued (per core, one Switch arm)
    ntrig = 0      # triggers fired
    nb0 = 0        # send0 broadcasts so far
    nb1 = 0

    def prep_step(t):
        """Enqueue broadcast descriptors for step t (pool, inside Switch)."""
        nonlocal nprep
        if not 0 <= t < T + LAG:
            return
        for arm in gp.Switch(pid, NC):
            gp.remote_dma_broadcast(
                hh[:, arm, (t % S), :], send0[:, t % 3, :],
                remote_sem=rsem0, local_sem=lsem0, rdests=rdests,
            ).then_inc(psem, 1)
        nprep += 1

    prep_step(0)

    # ============ helpers ============
    def cell_mms(ell, t, par, it):
        """PE: gate matmuls for layer ell step t (iteration it)."""
        W = wh0_sb if ell == 0 else wh1_sb
        U = U0_sb if ell == 0 else U1_sb
        ps = (ps0 if ell == 0 else ps1)[par]
        rs = rsem0
        co = 0 if ell == 0 else B
        ci, si = t // L, t % L
        slot = (it - 1) % S
        fns = []
        for m in range(MT):
            o = ps[:, m * B: (m + 1) * B]
            if t > 0:
                for k in range(KH):
                    fns.append(lambda o=o, m=m, k=k: pe.matmul(
                        o, W[:, k, m, :], hh[:, k, slot, co:co + B],
                        start=(k == 0), stop=False))
                fns.append(lambda o=o, m=m: pe.matmul(
                    o, id_sb[:],
                    U[:, ci % 2, m, si * B: (si + 1) * B],
                    start=False, stop=True))
            else:
                fns.append(lambda o=o, m=m: pe.matmul(
                    o, id_sb[:],
                    U[:, ci % 2, m, si * B: (si + 1) * B],
                    start=True, stop=True))
        extra = []
        if t > 0:
            extra.append((rs, 16 * it))
        reads = [f"wh{ell}", "id"] + [f"U{ell}_{ci % 2}m{m}" for m in range(MT)]
        dep.group("pe", fns, reads=reads, writes=[f"ps{ell}p{par}"],
                  extra=extra)

    def cell_post(ell, t, par, it):
        """ACT+DVE: activations, state update, send write for layer ell."""
        ps = (ps0 if ell == 0 else ps1)[par]
        Ss = (S0s if ell == 0 else S1s)
        Tgs = (Tg0s if ell == 0 else Tg1s)
        cns = (cn0s if ell == 0 else cn1s)
        Tcs = (Tc0s if ell == 0 else Tc1s)
        tp = tmpA if ell == 0 else tmpB
        cst = c0_t if ell == 0 else c1_t
        hst = h0_t if ell == 0 else h1_t
        snd = send0
        lsm = lsem0
        nb = nb0
        e = str(ell)
        pn = f"ps{e}p{par}"
        # ACT: sigmoid(i,f,o), tanh(g)
        dep.emit("act", lambda: act.activation(
            Ss[:, par, :], ps[:, 0:3 * B], AF.Sigmoid),
            reads=[pn], writes=[f"S{e}{par}"])
        dep.emit("act", lambda: act.activation(
            Tgs[:, par, :], ps[:, 3 * B:4 * B], AF.Tanh),
            reads=[pn], writes=[f"Tg{e}{par}"])
        # DVE chain
        dep.emit("dve", lambda: dve.tensor_scalar_mul(
            tp[:, 0, :], cst[:], 0.1), reads=[f"c{e}"], writes=[f"c01{e}"])
        dep.emit("dve", lambda: dve.tensor_scalar_mul(
            tp[:, 1, :], hst[:], 0.1), reads=[f"h{e}"], writes=[f"h01{e}"])
        dep.emit("dve", lambda: dve.tensor_scalar_mul(
            tp[:, 2, :], Ss[:, par, 2 * B:3 * B], 0.9),
            reads=[f"S{e}{par}"], writes=[f"So9{e}"])
        dep.emit("dve", lambda: dve.tensor_mul(
            tp[:, 3, :], Ss[:, par, B:2 * B], cst[:]),
            reads=[f"S{e}{par}", f"c{e}"], writes=[f"R{e}"])
        dep.emit("dve", lambda: dve.tensor_mul(
            tp[:, 4, :], Ss[:, par, 0:B], Tgs[:, par, :]),
            reads=[f"S{e}{par}", f"Tg{e}{par}"], writes=[f"Pi{e}"])
        dep.emit("dve", lambda: dve.tensor_add(
            cns[:, par, :], tp[:, 3, :], tp[:, 4, :]),
            reads=[f"R{e}", f"Pi{e}"], writes=[f"cn{e}{par}"])
        dep.emit("dve", lambda: dve.scalar_tensor_tensor(
            cst[:], cns[:, par, :], 0.9, tp[:, 0, :],
            op0=ALU.mult, op1=ALU.add),
            reads=[f"cn{e}{par}", f"c01{e}"], writes=[f"c{e}"])
        dep.emit("act", lambda: act.activation(
            Tcs[:, par, :], cns[:, par, :], AF.Tanh),
            reads=[f"cn{e}{par}"], writes=[f"Tc{e}{par}"])
        dep.emit("dve", lambda: dve.tensor_mul(
            tp[:, 5, :], tp[:, 2, :], Tcs[:, par, :]),
            reads=[f"So9{e}", f"Tc{e}{par}"], writes=[f"Hz{e}"])
        dep.emit("dve", lambda: dve.tensor_add(
            hst[:], tp[:, 5, :], tp[:, 1, :]),
            reads=[f"Hz{e}", f"h01{e}"], writes=[f"h{e}"])
        # send copy (bf16); WAR vs broadcast it-3 reading this slot
        sp3 = it % 3
        co = 0 if ell == 0 else B
        extra = []
        if nb >= 2:
            extra.append((lsm, 16 * (nb - 1)))
        dep.emit("dve", lambda: dve.tensor_copy(
            snd[:, sp3, co:co + B], hst[:]),
            reads=[f"h{e}"], writes=[f"send{e}{sp3}"], extra=extra)

    def do_trigger(t):
        nonlocal ntrig, nb0
        par = t % 3
        ntrig += 1
        nb0 += 1
        extra = [(psem, ntrig)]
        for e in ("0", "1"):
            tok = dep.lastw.get(f"send{e}{par}")
            if tok:
                extra.append(tok)
        dep.emit("gp", lambda: gp.trigger_dma(count=1), extra=extra, inc=False)
        for e in ("0", "1"):
            if f"send{e}{par}" in dep.lastw:
                dep.note_dma_read(f"send{e}{par}", lsem0, 16 * nb0)

    def emit_u0(c):
        lc = ch[c][1]
        nco = lc * B
        xp = c % 2
        for m in range(MT):
            pu = psU[m % 2]
            fns = []
            for k in range(KIN):
                fns.append(lambda m=m, k=k, pu=pu: pe.matmul(
                    pu[:, :nco], w0_sb[:, k, m, :],
                    xin_sb[:, xp, k, :nco],
                    start=(k == 0), stop=(k == KIN - 1)))
            dep.group("pe", fns, reads=["w0", f"xin{xp}"],
                      writes=[f"psU{m % 2}"])
            dep.emit("act", lambda m=m, pu=pu: act.copy(
                U0_sb[:, c % 2, m, :nco], pu[:, :nco]),
                reads=[f"psU{m % 2}"], writes=[f"U0_{c % 2}m{m}"])
        # prefetch xin chunk c+2 into slot (c+2)%2 (c, c+1 already resident)
        if c + 2 < len(ch):
            lcn = ch[c + 2][1]
            nonlocal_ndma_xin(c + 2, lcn)

    def nonlocal_ndma_xin(cc, lcn):
        nonlocal ndma
        # WAR: U0 GEMM of chunk cc-2 read this slot
        rd = dep.readers.get(f"xin{cc % 2}", ())
        for sem, val in rd:
            sp.wait_ge(sem, val)
        ndma += 1
        sp.dma_start(
            xin_sb[:, cc % 2, :, : lcn * B],
            xinT_d[:, :, cc * L * B: cc * L * B + lcn * B],
        ).then_inc(dmas, 16)
        dep.note_remote_write(f"xin{cc % 2}", dmas, 16 * ndma)

    def emit_u1(c):
        lc = ch[c][1]
        nco = lc * B
        lo = (c * L) % S
        for m in range(MT):
            pu = psU[m % 2]
            fns = []
            for k in range(KH + 1):
                rhs = hh[:, k, lo: lo + lc, 0:B]
                fns.append(lambda m=m, k=k, pu=pu, rhs=rhs: pe.matmul(
                    pu[:, :nco], w1_sb[:, k, m, :], rhs,
                    start=(k == 0), stop=(k == KH)))
            dep.group("pe", fns, reads=["w1", "h0ones"],
                      writes=[f"psU{m % 2}"],
                      extra=[(rsem0, 16 * (c * L + lc))])
            dep.emit("act", lambda m=m, pu=pu: act.copy(
                U1_sb[:, c % 2, m, :nco], pu[:, :nco]),
                reads=[f"psU{m % 2}"], writes=[f"U1_{c % 2}m{m}"])

    def emit_proj(c):
        nonlocal ndma
        lc = ch[c][1]
        nco = lc * B
        # h1 of step tau was sent at iteration tau+LAG -> slot (tau+LAG)%S
        lo = (c * L + LAG) % S
        segs = []
        if lo + lc <= S:
            segs.append((lo, lc, 0))
        else:
            segs.append((lo, S - lo, 0))
            segs.append((0, lc - (S - lo), S - lo))
        fns = []
        for (sl, ln, off) in segs:
            for k in range(KH + 1):
                rhs = hh[:, k, sl: sl + ln, B:2 * B]
                fns.append(lambda k=k, rhs=rhs, off=off, ln=ln:
                           pe.matmul(
                               psP[:, off * B: off * B + ln * B],
                               pj_sb[:, k, :], rhs,
                               start=(k == 0), stop=(k == KH)))
        dep.group("pe", fns, reads=["pj", "h1ones"], writes=["psP"],
                  extra=[(rsem0, 16 * (c * L + lc + LAG))])
        # WAR: previous y DMA from this ysb slot must be done
        extra = []
        if c >= 2:
            extra.append((dmay, 16 * (c - 1)))
        dep.emit("act", lambda: act.copy(
            ysb[:, c % 2, :nco], psP[:, :nco]),
            reads=["psP"], writes=[f"ysb{c % 2}"], extra=extra)
        sem, val = dep.lastw[f"ysb{c % 2}"]
        sp.wait_ge(sem, val)
        sp.dma_start(
            y_d[:, c * L * B: c * L * B + nco], ysb[:, c % 2, :nco]
        ).then_inc(dmay, 16)
        dep.note_dma_read(f"ysb{c % 2}", dmay, 16 * (c + 1))

    # ================= main loop =================
    emit_u0(0)
    for t in range(T + LAG):
        par = t % 2
        tau = t - LAG
        if t < T:
            cell_mms(0, t, par, t)
        if tau >= 0:
            cell_mms(1, tau, par, t)
        if t < T:
            cell_post(0, t, par, t)
        if tau >= 0:
            cell_post(1, tau, par, t)
        do_trigger(t)
        prep_step(t + 1)
        if t in u0_at:
            emit_u0(u0_at[t])
        if t in u1_at:
            emit_u1(u1_at[t])
        if t in pj_at:
            emit_proj(pj_at[t])

    for t_late in sorted(pj_at):
        if t_late >= T + LAG:
            emit_proj(pj_at[t_late])

    # final drains: all our sends flushed, all y DMAs done
    gp.wait_ge(lsem0, 16 * nb0)
    sp.wait_ge(dmay, 16 * len(ch))

    nc.compile()
    return nc


# ---------------- host-side data prep (same as baseline) ----------------
HU = H // NC


def _gate_rows(k):
    u = np.arange(k * HU, (k + 1) * HU)
    return np.concatenate([u, H + u, 3 * H + u, 2 * H + u])  # i, f, o, g


def _lhsT_blocks(w, nk, mt=MT):
    a = w.reshape(mt, P, nk, P)
    return np.ascontiguousarray(a.transpose(3, 2, 0, 1))


def prep_inputs(inputs, T):
    x = np.asarray(inputs["x"], np.float32)[:, :, :T]
    cond = np.asarray(inputs["cond"], np.float32)[:, :, :T]
    w_ih0 = np.asarray(inputs["w_ih0"], np.float32)
    w_hh0 = np.asarray(inputs["w_hh0"], np.float32)
    b0 = np.asarray(inputs["b_ih0"], np.float32) + np.asarray(inputs["b_hh0"], np.float32)
    w_ih1 = np.asarray(inputs["w_ih1"], np.float32)
    w_hh1 = np.asarray(inputs["w_hh1"], np.float32)
    b1 = np.asarray(inputs["b_ih1"], np.float32) + np.asarray(inputs["b_hh1"], np.float32)
    proj_w = np.asarray(inputs["proj_w"], np.float32)
    proj_b = np.asarray(inputs["proj_b"], np.float32)

    TB = T * B
    in0 = D + C
    xs = np.concatenate([np.zeros((B, D, 1), np.float32), x[:, :, : T - 1]], axis=2)
    xin = np.concatenate([xs, cond], axis=1)
    xin_pad = np.zeros((B, KIN * P, T), np.float32)
    xin_pad[:, :in0] = xin
    xin_pad[:, in0] = 1.0
    xinT = np.ascontiguousarray(xin_pad.transpose(1, 2, 0)).reshape(KIN * P, TB)
    xinT = np.ascontiguousarray(
        xinT.reshape(KIN, P, TB).transpose(1, 0, 2)
    ).astype(NP_BF16)

    w_ih0_pad = np.zeros((4 * H, KIN * P), np.float32)
    w_ih0_pad[:, :in0] = w_ih0
    w_ih0_pad[:, in0] = b0

    ident = np.eye(P, dtype=NP_BF16)

    in_maps = []
    for k in range(NC):
        r = _gate_rows(k)
        w0T = _lhsT_blocks(w_ih0_pad[r], KIN).astype(NP_BF16)
        wh0T = _lhsT_blocks(w_hh0[r], KH).astype(NP_BF16)
        w1_ext = np.zeros((MT * P, (KH + 1) * P), np.float32)
        w1_ext[:, : KH * P] = w_ih1[r]
        w1_ext[:, KH * P] = b1[r]
        w1T = _lhsT_blocks(w1_ext, KH + 1).astype(NP_BF16)
        wh1T = _lhsT_blocks(w_hh1[r], KH).astype(NP_BF16)
        pjT = np.zeros((P, KH + 1, PJP), np.float32)
        rows = np.arange(k * PJ, (k + 1) * PJ)
        for kk in range(KH):
            pjT[:, kk, :PJ] = proj_w[rows, kk * P: (kk + 1) * P].T
        pjT[0, KH, :PJ] = proj_b[rows]
        in_maps.append(
            {
                "xinT": xinT,
                "w0T": w0T,
                "wh0T": wh0T,
                "w1T": w1T,
                "wh1T": wh1T,
                "pjT": pjT.astype(NP_BF16),
                "ident": ident,
            }
        )
    return in_maps


def assemble(results, x_lengths, T):
    y = np.concatenate([r["y_out"][:PJ] for r in results], axis=0)
    y = y.reshape(D, T, B).transpose(2, 0, 1)
    lens = np.asarray(x_lengths).astype(np.int64)
    mask = (np.arange(T)[None, :] < lens[:, None]).astype(np.float32)
    return np.ascontiguousarray(y * mask[:, None, :])


_NC_CACHE = {}


def run(inputs, T=T_FULL, trace=False, **kw):
    if T not in _NC_CACHE:
        _NC_CACHE[T] = build_nc(T)
    nc = _NC_CACHE[T]
    in_maps = prep_inputs(inputs, T)
    res = bass_utils.run_bass_kernel_spmd(
        nc, in_maps, core_ids=list(range(NC)), trace=trace, **kw
    )
    out = assemble(res.results, inputs["x_lengths"], T)
    return out, res


def kernel(**inputs) -> np.ndarray:
    out, _ = run(inputs, T=T_FULL)
    return out
